# revision 1
# baseline (speedup 1.0000x reference)
"""Trainium2 Bass kernel for the PCNN (piecewise-CNN) bag-classification model.

Pipeline (per NeuronCore, data-parallel over sentences, 256 sentences/core):
  1. indirect-DMA gather of word/positional embeddings (token-major, bf16)
  2. PE transposes -> channel-major X tiles
  3. conv1d(k=3, edge-pad) as PSUM-accumulated matmuls over (tap, channel-chunk)
  4. PCNN piecewise max-pool: rank-1 mask matmuls into PSUM + serial reduce_max
  5. ReLU (+conv-bias fold), dense projection to 53 logits per sentence
  6. bag segment-mean as matmul with a host-built normalized selection matrix
  7. AllReduce over 8 cores, softmax, output [256, 53]

Scaling fold (exact reformulation): conv_w,conv_b are pre-scaled by 0.01 and
dense_w by 100 so the +100*mask trick of the reference becomes +1.0*mask,
keeping everything well-conditioned in bf16/fp32.
"""

import os
import sys

for _p in ("/opt/trn_rl_repo",):
    if _p not in sys.path:
        sys.path.insert(0, _p)

import numpy as np
import ml_dtypes

# ---------------- problem constants (hardcoded per spec) ----------------
N = 2048          # total sentences
L = 120           # max sentence length
LP = 122          # edge-padded length
NCORES = 8
NS = N // NCORES  # 256 sentences per core
BLK = 32          # sentences per block (SBUF-resident unit)
NBLK = NS // BLK  # 8 blocks
SGS = 4           # sentences per matmul subgroup
SG_PER_BLK = BLK // SGS          # 8
SG_COLS = 512                    # padded columns per subgroup (4*122=488 real)
BLK_COLS = SG_PER_BLK * SG_COLS  # 4096
TILES_PER_BLK = BLK_COLS // 128  # 32
NF = 230
NREL = 53
NBAGS = 256
VOCAB = 100000
WD = 300
PD = 5
NPOS = 240
FCH = [(0, 128), (128, 102)]          # filter chunks
CCH = [(0, 128), (128, 128), (256, 54)]  # channel chunks over [word(300), pf1(5), pf2(5)]

BF16 = ml_dtypes.bfloat16

_PROGRAM = None  # cached (nc,) across calls
LAST_RESULT = None


def _build_program():
    import concourse.bass as bass
    import concourse.mybir as mybir
    import concourse.tile as tile
    from concourse import bacc
    from concourse.masks import make_identity

    f32 = mybir.dt.float32
    bf16 = mybir.dt.bfloat16
    i32 = mybir.dt.int32
    AF = mybir.ActivationFunctionType
    AX = mybir.AxisListType

    nc = bacc.Bacc(
        "TRN2",
        target_bir_lowering=False,
        debug=False,
        num_devices=NCORES,
    )

    # ------------- external I/O -------------
    wemb = nc.dram_tensor("wemb", [VOCAB, WD], bf16, kind="ExternalInput").ap()
    xpf_d = nc.dram_tensor("xpf", [NBLK, 84, BLK_COLS], bf16, kind="ExternalInput").ap()
    idxw_d = nc.dram_tensor("idxw", [128, NBLK * TILES_PER_BLK], i32, kind="ExternalInput").ap()
    masks_d = nc.dram_tensor("masksd", [NBLK, 128, BLK * L], bf16, kind="ExternalInput").ap()
    snorm_d = nc.dram_tensor("snorm", [NS, NBAGS], bf16, kind="ExternalInput").ap()
    wt_d = nc.dram_tensor("wt", [3, 128, 3 * NF], bf16, kind="ExternalInput").ap()
    dwt_d = nc.dram_tensor("dwt", [128, 6 * NREL], bf16, kind="ExternalInput").ap()
    actb_d = nc.dram_tensor("actb", [128, 2], f32, kind="ExternalInput").ap()
    dbias_d = nc.dram_tensor("dbias", [1, NREL], bf16, kind="ExternalInput").ap()
    out_d = nc.dram_tensor("out", [NBAGS, NREL], f32, kind="ExternalOutput").ap()
    debug = bool(int(os.environ.get("KERNEL_DEBUG", "0")))
    if debug:
        dbg_xg = nc.dram_tensor("dbg_xg", [128, TILES_PER_BLK, WD + 2 * PD], bf16,
                                kind="ExternalOutput").ap()
        dbg_xc = nc.dram_tensor("dbg_xc", [3, 128, BLK_COLS], bf16,
                                kind="ExternalOutput").ap()
        dbg_pooled = nc.dram_tensor("dbg_pooled", [2, 128, 3, NS], f32,
                                    kind="ExternalOutput").ap()
        dbg_bag = nc.dram_tensor("dbg_bag", [NBAGS, NREL], f32,
                                 kind="ExternalOutput").ap()

    with tile.TileContext(nc) as tc:
        import contextlib

        ctx = contextlib.ExitStack()
        with ctx:
            singles = ctx.enter_context(tc.tile_pool(name="singles", bufs=1))

            # persistent tiles
            wt_sb = [singles.tile([128, 3 * NF], bf16, name=f"wt{c}") for c in range(3)]
            sel = [singles.tile([128, 128], bf16, name=f"sel{j}") for j in range(3)]
            snorm_sb = [singles.tile([128, NBAGS], bf16, name=f"sn{c}") for c in range(2)]
            idxw_sb = singles.tile([128, NBLK * TILES_PER_BLK], i32)
            dwt_sb = singles.tile([128, 6 * NREL], bf16)
            actb_sb = singles.tile([128, 2], f32)
            dbias_sb = singles.tile([1, NREL], bf16)
            ident = singles.tile([128, 128], bf16)
            ones_sb = singles.tile([1, 128], bf16)
            pooled = [singles.tile([128, 3, NS], f32, name=f"pool{c}") for c in range(2)]

            for c in range(3):
                nc.sync.dma_start(out=wt_sb[c][:, :], in_=wt_d[c, :, :])
            for c in range(2):
                nc.sync.dma_start(out=snorm_sb[c][:, :], in_=snorm_d[c * 128:(c + 1) * 128, :])
            nc.sync.dma_start(out=idxw_sb[:, :], in_=idxw_d[:, :])
            nc.sync.dma_start(out=dwt_sb[:, :], in_=dwt_d[:, :])
            nc.sync.dma_start(out=actb_sb[:, :], in_=actb_d[:, :])
            nc.sync.dma_start(out=dbias_sb[:, :], in_=dbias_d[:, :])
            make_identity(nc, ident[:, :])
            pidx = singles.tile([128, 128], mybir.dt.int32, name="pidx")
            nc.gpsimd.iota(pidx[:, :], pattern=[[0, 128]], base=0, channel_multiplier=1)
            for j in range(3):
                nc.vector.tensor_scalar(
                    out=sel[j][:, :], in0=pidx[:, :], scalar1=j, scalar2=None,
                    op0=mybir.AluOpType.is_equal,
                )
            nc.vector.memset(ones_sb[:, :], 1.0)
            nc.vector.memset(pooled[0][:, :, :], 0.0)
            nc.vector.memset(pooled[1][:, :, :], 0.0)

            xg_pool = ctx.enter_context(tc.tile_pool(name="xg", bufs=3))
            mask_pool = ctx.enter_context(tc.tile_pool(name="mask", bufs=2))
            xc_pool = ctx.enter_context(tc.tile_pool(name="xc", bufs=2))
            tp_psum = ctx.enter_context(tc.tile_pool(name="tp", bufs=2, space="PSUM"))
            cv_psum = ctx.enter_context(tc.tile_pool(name="cv", bufs=6, space="PSUM"))

            pending = None
            for blk in range(NBLK):
                # ---- gather (token-major) ----
                xg = xg_pool.tile([128, TILES_PER_BLK, WD], bf16, tag="xg")
                for t in range(TILES_PER_BLK):
                    col = blk * TILES_PER_BLK + t
                    nc.gpsimd.indirect_dma_start(
                        out=xg[:, t, 0:WD],
                        out_offset=None,
                        in_=wemb[:, :],
                        in_offset=bass.IndirectOffsetOnAxis(
                            ap=idxw_sb[:, col:col + 1], axis=0),
                    )
                mask_sb = mask_pool.tile([128, BLK * L], bf16, tag="mask")
                nc.sync.dma_start(out=mask_sb[:, :], in_=masks_d[blk, :, :])

                if debug and blk == 0:
                    nc.sync.dma_start(out=dbg_xg[:, :, :], in_=xg[:, :, :])
                # ---- transpose to channel-major ----
                xc = [
                    xc_pool.tile([128, BLK_COLS], bf16, tag=f"xc{c}", name=f"xc{c}")
                    for c in range(3)
                ]
                nc.sync.dma_start(out=xc[2][44:128, :], in_=xpf_d[blk, :, :])
                for grp in range(4):  # 8 token-tiles per group
                    for cc, (c0, cw) in enumerate(CCH):
                        pw = cw if cc < 2 else 44
                        tpA = tp_psum.tile([128, 4, 128], bf16, tag="tp", name=f"tpA{cc}")
                        tpB = tp_psum.tile([128, 4, 128], bf16, tag="tp", name=f"tpB{cc}")
                        for t in range(8):
                            ti = grp * 8 + t
                            tgt = tpA if t % 2 == 0 else tpB
                            nc.tensor.transpose(
                                out=tgt[0:pw, t // 2, :],
                                in_=xg[:, ti, c0:c0 + pw],
                                identity=ident[:, :],
                            )
                        for half, tp in ((0, tpA), (1, tpB)):
                            cb = xc[cc][0:pw, grp * 1024 + half * 128:
                                        grp * 1024 + half * 128 + 1]
                            dst = bass.AP(
                                tensor=cb.tensor, offset=cb.offset,
                                ap=[cb.ap[0], [256, 4], [1, 128]],
                            )
                            nc.scalar.copy(out=dst, in_=tp[0:pw, :, :])

                if debug and blk == 0:
                    for c in range(3):
                        nc.sync.dma_start(out=dbg_xc[c, :, :], in_=xc[c][:, :])
                # ---- conv + piecewise max-pool (software-pipelined x3 units) ----
                units = [(sg, fc) for sg in range(SG_PER_BLK) for fc in range(2)]
                unit_groups = [units[g:g + 3] for g in range(0, len(units), 3)]

                def emit_conv(grp_units, tiles):
                    for (sg, fc) in grp_units:
                        f0, fw = FCH[fc]
                        ps = cv_psum.tile([128, SGS, L], f32, tag="cv",
                                          name=f"cv{sg}_{fc}")
                        tiles[(sg, fc)] = ps
                        nmm = 0
                        for k in range(3):
                            for cc in range(3):
                                base = xc[cc][0:128, sg * SG_COLS + k:sg * SG_COLS + k + 1]
                                rhs = bass.AP(
                                    tensor=base.tensor,
                                    offset=base.offset,
                                    ap=[base.ap[0], [LP, SGS], [1, L]],
                                )
                                nc.tensor.matmul(
                                    out=ps[0:fw, :, :],
                                    lhsT=wt_sb[cc][0:128, k * NF + f0:k * NF + f0 + fw],
                                    rhs=rhs,
                                    start=(nmm == 0),
                                    stop=False,
                                    skip_group_check=True,
                                )
                                nmm += 1

                def emit_jphases(grp_units, tiles, mask_sb=None):
                    for j in range(3):
                        for (sg, fc) in grp_units:
                            f0, fw = FCH[fc]
                            ps = tiles[(sg, fc)]
                            s0 = tiles[("blk", sg, fc)] * BLK + sg * SGS
                            nc.vector.reduce_max(
                                out=pooled[fc][0:fw, j, s0:s0 + SGS],
                                in_=ps[0:fw, :, :],
                                axis=AX.X,
                            )
                            if j < 2:
                                nc.tensor.matmul(
                                    out=ps[0:fw, :, :],
                                    lhsT=sel[j + 1][:, 0:fw],
                                    rhs=tiles[("mask", sg, fc)][
                                        :, sg * SGS * L:(sg + 1) * SGS * L],
                                    start=False,
                                    stop=(j == 1),
                                    skip_group_check=True,
                                )

                for grp_units in unit_groups:
                    tiles = {}
                    for (sg, fc) in grp_units:
                        tiles[("blk", sg, fc)] = blk
                        tiles[("mask", sg, fc)] = mask_sb
                    emit_conv(grp_units, tiles)
                    if pending is not None:
                        emit_jphases(*pending)
                    pending = (grp_units, tiles)

            if pending is not None:
                emit_jphases(*pending)
                pending = None

            # ---------------- tail ----------------
            if debug:
                for fc in range(2):
                    nc.sync.dma_start(out=dbg_pooled[fc, :, :, :], in_=pooled[fc][:, :, :])
            # ReLU(max - 1 + 0.01*conv_b), cast to bf16
            pr = [singles.tile([128, 3, NS], bf16, name=f"pr{c}") for c in range(2)]
            for fc in range(2):
                nc.scalar.activation(
                    out=pr[fc][:, :, :],
                    in_=pooled[fc][:, :, :],
                    func=AF.Relu,
                    bias=actb_sb[:, fc:fc + 1],
                    scale=1.0,
                )

            # dense: logitsT [53, 256] = sum_{j,fc} dwt[(j,fc)].T @ pooled_r
            lg_ps = cv_psum.tile([NREL, NS], f32, tag="cv", name="lgps")
            nmm = 0
            for j in range(3):
                for fc, (f0, fw) in enumerate(FCH):
                    nc.tensor.matmul(
                        out=lg_ps[:, :],
                        lhsT=dwt_sb[0:fw, (j * 2 + fc) * NREL:(j * 2 + fc + 1) * NREL],
                        rhs=pr[fc][0:fw, j, :],
                        start=(nmm == 0),
                        stop=(nmm == 5),
                    )
                    nmm += 1
            lg_sb = singles.tile([NREL, NS], bf16)
            nc.vector.tensor_copy(out=lg_sb[:, :], in_=lg_ps[:, :])

            # transpose logits -> [256 sents, 53]
            ls = [singles.tile([128, NREL], bf16, name=f"ls{c}") for c in range(2)]
            for sc in range(2):
                ltp = tp_psum.tile([128, 4, 128], bf16, tag="tp", name="ltp")
                nc.tensor.transpose(
                    out=ltp[0:128, 0, 0:NREL],
                    in_=lg_sb[:, sc * 128:(sc + 1) * 128],
                    identity=ident[0:NREL, 0:NREL],
                )
                nc.vector.tensor_copy(out=ls[sc][:, :], in_=ltp[0:128, 0, 0:NREL])

            # bag aggregation: bagT [128 bags, 53] per bag-chunk (+ dense bias/8)
            cc_dram = ctx.enter_context(tc.tile_pool(name="ccd", bufs=1, space="DRAM"))
            cc_in = cc_dram.tile([NBAGS, NREL], f32)
            cc_out = cc_dram.tile([NBAGS, NREL], f32)
            for bc in range(2):
                bg = cv_psum.tile([128, NREL], f32, tag="cv", name="bg")
                for sc in range(2):
                    nc.tensor.matmul(
                        out=bg[:, :],
                        lhsT=snorm_sb[sc][:, bc * 128:(bc + 1) * 128],
                        rhs=ls[sc][:, :],
                        start=(sc == 0),
                        stop=False,
                    )
                nc.tensor.matmul(
                    out=bg[:, :],
                    lhsT=ones_sb[0:1, 0:128],
                    rhs=dbias_sb[0:1, :],
                    start=False,
                    stop=True,
                )
                bg_sb = singles.tile([128, NREL], f32, name=f"bgs{bc}")
                nc.vector.tensor_copy(out=bg_sb[:, :], in_=bg[:, :])
                nc.sync.dma_start(out=cc_in[bc * 128:(bc + 1) * 128, :], in_=bg_sb[:, :])

            if debug:
                nc.sync.dma_start(out=dbg_bag[:, :], in_=cc_in[:, :])
            nc.gpsimd.collective_compute(
                "AllReduce",
                mybir.AluOpType.add,
                replica_groups=[list(range(NCORES))],
                ins=[cc_in.opt()],
                outs=[cc_out.opt()],
            )

            # softmax over the 53 relations
            for bc in range(2):
                t = singles.tile([128, NREL], f32, name=f"sm{bc}")
                nc.sync.dma_start(out=t[:, :], in_=cc_out[bc * 128:(bc + 1) * 128, :])
                nmax = singles.tile([128, 1], f32, name=f"nmax{bc}")
                nc.vector.reduce_max(out=nmax[:, :], in_=t[:, :], axis=AX.X, negate=True)
                ex = singles.tile([128, NREL], f32, name=f"ex{bc}")
                nc.scalar.activation(
                    out=ex[:, :], in_=t[:, :], func=AF.Exp, bias=nmax[:, :], scale=1.0
                )
                ssum = singles.tile([128, 1], f32, name=f"ssum{bc}")
                nc.vector.reduce_sum(out=ssum[:, :], in_=ex[:, :], axis=AX.X)
                rcp = singles.tile([128, 1], f32, name=f"rcp{bc}")
                nc.vector.reciprocal(out=rcp[:, :], in_=ssum[:, :])
                res = singles.tile([128, NREL], f32, name=f"res{bc}")
                nc.vector.tensor_scalar_mul(res[:, :], ex[:, :], rcp[:, :])
                nc.sync.dma_start(out=out_d[bc * 128:(bc + 1) * 128, :], in_=res[:, :])

    nc.compile()
    return nc


def _get_program():
    global _PROGRAM
    if _PROGRAM is None:
        _PROGRAM = _build_program()
    return _PROGRAM


def _pad_edge(a):
    return np.concatenate([a[:, :1], a, a[:, -1:]], axis=1)


def _token_layout(padded):
    """[NS, LP] int32 -> gather-index layout [128, NBLK*TILES_PER_BLK].

    Within each block: 8 subgroups of 4 sentences, each padded to 512 cols
    (pad index 0). idx[p, blk*32+i] = stream[blk][i*128+p]."""
    a = padded.reshape(NBLK, SG_PER_BLK, SGS * LP)
    tok = np.zeros((NBLK, SG_PER_BLK, SG_COLS), np.int32)
    tok[:, :, :SGS * LP] = a
    flat = tok.reshape(NBLK, TILES_PER_BLK, 128)
    return flat.transpose(2, 0, 1).reshape(128, NBLK * TILES_PER_BLK)


def kernel(**inputs):
    sentences = np.asarray(inputs["sentences"]).astype(np.int32)
    pos1 = np.asarray(inputs["pos1"]).astype(np.int32)
    pos2 = np.asarray(inputs["pos2"]).astype(np.int32)
    masks = np.asarray(inputs["masks"]).astype(np.float32)
    bag_ids = np.asarray(inputs["bag_ids"]).astype(np.int64)
    word_emb = np.asarray(inputs["word_emb"]).astype(np.float32)
    pf1_emb = np.asarray(inputs["pf1_emb"]).astype(np.float32)
    pf2_emb = np.asarray(inputs["pf2_emb"]).astype(np.float32)
    conv_w = np.asarray(inputs["conv_w"]).astype(np.float32)
    conv_b = np.asarray(inputs["conv_b"]).astype(np.float32)
    dense_w = np.asarray(inputs["dense_w"]).astype(np.float32)
    dense_b = np.asarray(inputs["dense_b"]).astype(np.float32)

    # ---- shared (replicated) parameter prep ----
    wemb_bf = word_emb.astype(BF16)

    w01 = (conv_w * 0.01).transpose(1, 0, 2)  # [310, 230, 3]
    wt = np.zeros((3, 128, 3 * NF), np.float32)
    for cc, (c0, cw) in enumerate(CCH):
        wt[cc, :cw, :] = w01[c0:c0 + cw].transpose(0, 2, 1).reshape(cw, 3 * NF)
    wt[2, 54, NF:2 * NF] = 1.0  # +mask_j0 rides the center tap via xc2 row 54
    wt = wt.astype(BF16)

    dw100 = dense_w * 100.0  # [53, 690]
    dwt = np.zeros((128, 6 * NREL), np.float32)
    for j in range(3):
        for fc, (f0, fw) in enumerate(FCH):
            dwt[:fw, (j * 2 + fc) * NREL:(j * 2 + fc + 1) * NREL] = \
                dw100[:, j * NF + f0:j * NF + f0 + fw].T
    dwt = dwt.astype(BF16)

    actb = np.full((128, 2), -1.0, np.float32)
    for fc, (f0, fw) in enumerate(FCH):
        actb[:fw, fc] = 0.01 * conv_b[f0:f0 + fw] - 1.0

    dbias = (dense_b / NCORES).reshape(1, NREL).astype(BF16)

    counts = np.bincount(bag_ids, minlength=NBAGS).astype(np.float32)
    counts = np.maximum(counts, 1.0)

    # ---- per-core prep ----
    in_maps = []
    for r in range(NCORES):
        sl = slice(r * NS, (r + 1) * NS)
        m = masks[sl]  # [256, 3, 120]
        md = np.stack([m[:, 0], m[:, 1] - m[:, 0], m[:, 2] - m[:, 1]], axis=1)
        idxw = _token_layout(_pad_edge(sentences[sl]))
        p1p = _pad_edge(pos1[sl])  # [NS, LP]
        p2p = _pad_edge(pos2[sl])
        pfv = np.concatenate([pf1_emb[p1p], pf2_emb[p2p]], axis=2)  # [NS, LP, 10]
        xpf = np.zeros((NBLK, SG_PER_BLK, SG_COLS, 2 * PD), np.float32)
        xpf[:, :, :SGS * LP, :] = pfv.reshape(NBLK, SG_PER_BLK, SGS * LP, 2 * PD)
        xpf10 = xpf.transpose(0, 3, 1, 2).reshape(NBLK, 2 * PD, BLK_COLS)
        xpf = np.zeros((NBLK, 84, BLK_COLS), np.float32)
        xpf[:, 0:2 * PD, :] = xpf10
        mj0 = np.zeros((NBLK, SG_PER_BLK, SG_COLS), np.float32)
        mj0v = mj0[:, :, :SGS * LP].reshape(NBLK, SG_PER_BLK, SGS, LP)
        mj0v[:, :, :, 1:L + 1] = md[:, 0, :].reshape(NBLK, SG_PER_BLK, SGS, L)
        xpf[:, 2 * PD, :] = mj0.reshape(NBLK, BLK_COLS)
        xpf = xpf.astype(BF16)

        masksd = np.zeros((NBLK, 128, BLK * L), np.float32)
        masksd[:, 0:3, :] = md.reshape(NBLK, BLK, 3, L).transpose(0, 2, 1, 3) \
                              .reshape(NBLK, 3, BLK * L)
        masksd = masksd.astype(BF16)

        bags = bag_ids[sl]
        snorm = np.zeros((NS, NBAGS), np.float32)
        snorm[np.arange(NS), bags] = 1.0 / counts[bags]
        snorm = snorm.astype(BF16)

        in_maps.append({
            "wemb": wemb_bf,
            "idxw": idxw.astype(np.int32),
            "xpf": xpf,
            "masksd": masksd,
            "snorm": snorm,
            "wt": wt,
            "dwt": dwt,
            "actb": actb,
            "dbias": dbias,
        })

    nc = _get_program()
    from concourse.bass_utils import run_bass_kernel_spmd

    trace = bool(int(os.environ.get("KERNEL_TRACE", "0")))
    res = run_bass_kernel_spmd(
        nc, in_maps, core_ids=list(range(NCORES)), trace=trace
    )
    global LAST_RESULT
    LAST_RESULT = res
    return res.results[0]["out"].astype(np.float32)


if __name__ == "__main__":
    d = np.load("/root/problem/ref_inputs.npz")
    out = kernel(**{k: d[k] for k in d.files})
    print("out", out.shape, out.dtype)
    ref = np.load("/root/problem/ref_out.npy")
    err = np.abs(out - ref).max() / np.abs(ref).max()
    print("Relative error:", err)



# revision 28
# speedup vs baseline: 1.1452x; 1.1452x over previous
"""Trainium2 Bass kernel for the PCNN (piecewise-CNN) bag-classification model.

V2 design (data-parallel over sentences, 256 sentences/core):
  Host: embedding gather + channel-major fp8(e4m3) layout upload (no on-device
        gather/transpose at all).
  Device per block of 32 sentences:
    conv1d(k=3, edge-pad) as fp8 DoubleRow matmuls: channels 0..255 ride the
    pair axis (q) of 3 full-K DR matmuls (one per tap); channels 256..309 + the
    piece-0 mask row ride a row-tiled triple (3 concurrent 28-pair DR matmuls,
    one per tap, at partition bases 0/32/64).
    PCNN piecewise max-pool: mask bias +128 (fp8-exact); j0 mask rides the conv
    contraction; j1/j2 are rank-1 fp8 matmul adds into PSUM; the three phase
    maxima come from 4-unit-batched DVE reduce_max over 4 PSUM banks.
  Tail: ReLU(+bias-128), dense to 53 logits, PE transpose, bag segment-mean as
        matmul with host-built normalized selection matrix, AllReduce, softmax.
"""

import os
import sys

for _p in ("/opt/trn_rl_repo",):
    if _p not in sys.path:
        sys.path.insert(0, _p)

import numpy as np
import ml_dtypes

# ---------------- problem constants (hardcoded per spec) ----------------
N = 2048          # total sentences
L = 120           # max sentence length
LP = 122          # edge-padded length
NCORES = 8
NS = N // NCORES  # 256 sentences per core
BLK = 32          # sentences per block
NBLK = NS // BLK  # 8 blocks
SGS = 4           # sentences per matmul unit
SG_PER_BLK = BLK // SGS          # 8
SG_COLS = 512                    # padded columns per unit (4*122=488 real)
BLK_COLS = SG_PER_BLK * SG_COLS  # 4096
NF = 230
NREL = 53
NBAGS = 256
VOCAB = 100000
WD = 300
PD = 5
IN_CH = WD + 2 * PD   # 310
FCH = [(0, 128), (128, 102)]   # filter chunks
MB = 128.0            # mask bias (fp8-exact)
NPAIR_B = 27          # channel pairs in chunk B (ch 256..309)
KB = NPAIR_B + 1      # + mask row

E4 = ml_dtypes.float8_e4m3fn
BF16 = ml_dtypes.bfloat16

_PROGRAM = None
LAST_RESULT = None


def _build_program():
    import concourse.bass as bass
    import concourse.mybir as mybir
    import concourse.tile as tile
    from concourse import bacc
    from concourse.masks import make_identity

    f32 = mybir.dt.float32
    bf16 = mybir.dt.bfloat16
    fp8 = mybir.dt.float8e4
    AF = mybir.ActivationFunctionType
    AX = mybir.AxisListType
    DR = mybir.MatmulPerfMode.DoubleRow

    nc = bacc.Bacc(
        "TRN2",
        target_bir_lowering=False,
        debug=False,
        num_devices=NCORES,
    )

    # ------------- external I/O -------------
    xa_d = nc.dram_tensor("xa", [NBLK, 128, 2, BLK_COLS], fp8, kind="ExternalInput").ap()
    xb_d = nc.dram_tensor("xb", [NBLK, KB, 2, BLK_COLS], fp8, kind="ExternalInput").ap()
    dm_d = nc.dram_tensor("dm", [NBLK, 1, 2 * BLK * L], fp8, kind="ExternalInput").ap()
    wa_d = nc.dram_tensor("wa", [128, 2, 3, 2, 128], fp8, kind="ExternalInput").ap()
    wb_d = nc.dram_tensor("wb", [KB, 2, 3, 2, 128], fp8, kind="ExternalInput").ap()
    snorm_d = nc.dram_tensor("snorm", [NS, NBAGS], bf16, kind="ExternalInput").ap()
    dwt_d = nc.dram_tensor("dwt", [128, 6 * NREL], bf16, kind="ExternalInput").ap()
    actb_d = nc.dram_tensor("actb", [128, 2], f32, kind="ExternalInput").ap()
    dbias_d = nc.dram_tensor("dbias", [1, NREL], bf16, kind="ExternalInput").ap()
    out_d = nc.dram_tensor("out", [NBAGS, NREL], f32, kind="ExternalOutput").ap()

    with tile.TileContext(nc) as tc:
        import contextlib

        ctx = contextlib.ExitStack()
        with ctx:
            singles = ctx.enter_context(tc.tile_pool(name="singles", bufs=1))

            # persistent tiles
            wa_sb = singles.tile([128, 2, 3, 2, 128], fp8, name="wa")
            wb_sb = singles.tile([KB, 2, 3, 2, 128], fp8, name="wb")
            snorm_sb = [singles.tile([128, NBAGS], bf16, name=f"sn{c}") for c in range(2)]
            dwt_sb = singles.tile([128, 6 * NREL], bf16)
            actb_sb = singles.tile([128, 2], f32)
            dbias_sb = singles.tile([1, NREL], bf16)
            ident = singles.tile([128, 128], bf16)
            ones_sb = singles.tile([1, 128], bf16)
            ones8 = singles.tile([1, 128], fp8)
            pooled = [singles.tile([128, 3, NS], f32, name=f"pool{c}") for c in range(2)]

            nc.sync.dma_start(out=wa_sb[:, :, :, :, :], in_=wa_d[:, :, :, :, :])
            nc.sync.dma_start(out=wb_sb[:, :, :, :, :], in_=wb_d[:, :, :, :, :])
            for c in range(2):
                nc.sync.dma_start(out=snorm_sb[c][:, :], in_=snorm_d[c * 128:(c + 1) * 128, :])
            nc.sync.dma_start(out=dwt_sb[:, :], in_=dwt_d[:, :])
            nc.sync.dma_start(out=actb_sb[:, :], in_=actb_d[:, :])
            nc.sync.dma_start(out=dbias_sb[:, :], in_=dbias_d[:, :])
            make_identity(nc, ident[:, :])
            nc.vector.memset(ones_sb[:, :], 1.0)
            nc.vector.memset(ones8[:, :], 1.0)
            nc.vector.memset(pooled[0][:, :, :], 0.0)
            nc.vector.memset(pooled[1][:, :, :], 0.0)

            xa_pool = ctx.enter_context(tc.tile_pool(name="xa", bufs=2))
            xb_pool = ctx.enter_context(tc.tile_pool(name="xb", bufs=2))
            dm_pool = ctx.enter_context(tc.tile_pool(name="dm", bufs=2))
            cv_psum = ctx.enter_context(tc.tile_pool(name="cv", bufs=2, space="PSUM"))

            for blk in range(NBLK):
                xa = xa_pool.tile([128, 2, BLK_COLS], fp8, tag="xa")
                xb = xb_pool.tile([KB, 2, BLK_COLS], fp8, tag="xb")
                dm = dm_pool.tile([1, 2 * BLK * L], fp8, tag="dm")
                nc.sync.dma_start(out=xa[:, :, :], in_=xa_d[blk, :, :, :])
                nc.sync.dma_start(out=xb[:, :, :], in_=xb_d[blk, :, :, :])
                nc.sync.dma_start(out=dm[:, :], in_=dm_d[blk, :, :])

                UL = SGS * L  # 480 interleaved output columns per unit
                for fc, (f0, fw) in enumerate(FCH):
                    for grp in range(2):  # 4 units each
                        ps = cv_psum.tile([128, 4, 512], f32, tag="cv",
                                          name=f"cv{fc}_{grp}")
                        # ---- conv: 3 full DR streams per unit ----
                        for tap in range(3):
                            lhsA = wa_sb[:, :, tap, fc, 0:fw]
                            for u in range(4):
                                sg = grp * 4 + u
                                base = xa[0:128, 0:2, sg * SG_COLS + SGS * tap:
                                          sg * SG_COLS + SGS * tap + 1]
                                rhs = bass.AP(
                                    tensor=base.tensor, offset=base.offset,
                                    ap=[base.ap[0], [BLK_COLS, 2], [1, UL]],
                                )
                                nc.tensor.matmul(
                                    out=ps[0:fw, u, 0:UL],
                                    lhsT=lhsA,
                                    rhs=rhs,
                                    start=(tap == 0),
                                    stop=False,
                                    perf_mode=DR,
                                    skip_group_check=True,
                                )
                        # ---- chunk B: 3 tap streams per unit ----
                        for t in range(3):
                            lhsB = wb_sb[0:KB, :, t, fc, 0:fw]
                            for u in range(4):
                                sg = grp * 4 + u
                                base = xb[0:KB, 0:2,
                                          sg * SG_COLS + SGS * t:
                                          sg * SG_COLS + SGS * t + 1]
                                rhs = bass.AP(
                                    tensor=base.tensor, offset=base.offset,
                                    ap=[base.ap[0], [BLK_COLS, 2], [1, UL]],
                                )
                                nc.tensor.matmul(
                                    out=ps[0:fw, u, 0:UL],
                                    lhsT=lhsB,
                                    rhs=rhs,
                                    start=False,
                                    stop=False,
                                    perf_mode=DR,
                                    skip_group_check=True,
                                )
                        # ---- piecewise max phases ----
                        s0 = blk * BLK + grp * 16
                        rbase = ps[0:fw, 0:4, 0:1]
                        rin = bass.AP(
                            tensor=rbase.tensor, offset=rbase.offset,
                            ap=[rbase.ap[0], [512, 4], [1, SGS], [SGS, L]],
                        )
                        for j in range(3):
                            nc.vector.reduce_max(
                                out=pooled[fc][0:fw, j, s0:s0 + 16],
                                in_=rin,
                                axis=AX.X,
                            )
                            if j < 2:
                                for u in range(4):
                                    sg = grp * 4 + u
                                    nc.tensor.matmul(
                                        out=ps[0:fw, u, 0:UL],
                                        lhsT=ones8[0:1, 0:fw],
                                        rhs=dm[0:1, j * BLK * L + sg * UL:
                                               j * BLK * L + (sg + 1) * UL],
                                        start=False,
                                        stop=(j == 1 and u == 3),
                                        skip_group_check=True,
                                    )

            # ---------------- tail ----------------
            pr = [singles.tile([128, 3, NS], bf16, name=f"pr{c}") for c in range(2)]
            for fc in range(2):
                nc.scalar.activation(
                    out=pr[fc][:, :, :],
                    in_=pooled[fc][:, :, :],
                    func=AF.Relu,
                    bias=actb_sb[:, fc:fc + 1],
                    scale=1.0,
                )

            # dense: logitsT [53, 256] = sum_{j,fc} dwt[(j,fc)].T @ pr
            lg_ps = cv_psum.tile([NREL, NS], f32, tag="cv", name="lgps")
            nmm = 0
            for j in range(3):
                for fc, (f0, fw) in enumerate(FCH):
                    nc.tensor.matmul(
                        out=lg_ps[:, :],
                        lhsT=dwt_sb[0:fw, (j * 2 + fc) * NREL:(j * 2 + fc + 1) * NREL],
                        rhs=pr[fc][0:fw, j, :],
                        start=(nmm == 0),
                        stop=(nmm == 5),
                    )
                    nmm += 1
            lg_sb = singles.tile([NREL, NS], bf16)
            nc.vector.tensor_copy(out=lg_sb[:, :], in_=lg_ps[:, :])

            # transpose logits -> [256 sents, 53]
            ls = [singles.tile([128, NREL], bf16, name=f"ls{c}") for c in range(2)]
            for sc in range(2):
                ltp = cv_psum.tile([128, 128], bf16, tag="cv", name="ltp")
                nc.tensor.transpose(
                    out=ltp[0:128, 0:NREL],
                    in_=lg_sb[:, sc * 128:(sc + 1) * 128],
                    identity=ident[0:NREL, 0:NREL],
                )
                nc.vector.tensor_copy(out=ls[sc][:, :], in_=ltp[0:128, 0:NREL])

            # bag aggregation: bagT [128 bags, 53] per bag-chunk (+ dense bias/8)
            cc_dram = ctx.enter_context(tc.tile_pool(name="ccd", bufs=1, space="DRAM"))
            cc_in = cc_dram.tile([NBAGS, NREL], f32)
            cc_out = cc_dram.tile([NBAGS, NREL], f32)
            for bc in range(2):
                bg = cv_psum.tile([128, NREL], f32, tag="cv", name="bg")
                for sc in range(2):
                    nc.tensor.matmul(
                        out=bg[:, :],
                        lhsT=snorm_sb[sc][:, bc * 128:(bc + 1) * 128],
                        rhs=ls[sc][:, :],
                        start=(sc == 0),
                        stop=False,
                    )
                nc.tensor.matmul(
                    out=bg[:, :],
                    lhsT=ones_sb[0:1, 0:128],
                    rhs=dbias_sb[0:1, :],
                    start=False,
                    stop=True,
                )
                bg_sb = singles.tile([128, NREL], f32, name=f"bgs{bc}")
                nc.vector.tensor_copy(out=bg_sb[:, :], in_=bg[:, :])
                nc.sync.dma_start(out=cc_in[bc * 128:(bc + 1) * 128, :], in_=bg_sb[:, :])

            nc.gpsimd.collective_compute(
                "AllReduce",
                mybir.AluOpType.add,
                replica_groups=[list(range(NCORES))],
                ins=[cc_in.opt()],
                outs=[cc_out.opt()],
            )

            # softmax over the 53 relations
            for bc in range(2):
                t = singles.tile([128, NREL], f32, name=f"sm{bc}")
                nc.sync.dma_start(out=t[:, :], in_=cc_out[bc * 128:(bc + 1) * 128, :])
                nmax = singles.tile([128, 1], f32, name=f"nmax{bc}")
                nc.vector.reduce_max(out=nmax[:, :], in_=t[:, :], axis=AX.X, negate=True)
                ex = singles.tile([128, NREL], f32, name=f"ex{bc}")
                nc.scalar.activation(
                    out=ex[:, :], in_=t[:, :], func=AF.Exp, bias=nmax[:, :], scale=1.0
                )
                ssum = singles.tile([128, 1], f32, name=f"ssum{bc}")
                nc.vector.reduce_sum(out=ssum[:, :], in_=ex[:, :], axis=AX.X)
                rcp = singles.tile([128, 1], f32, name=f"rcp{bc}")
                nc.vector.reciprocal(out=rcp[:, :], in_=ssum[:, :])
                res = singles.tile([128, NREL], f32, name=f"res{bc}")
                nc.vector.tensor_scalar_mul(res[:, :], ex[:, :], rcp[:, :])
                nc.sync.dma_start(out=out_d[bc * 128:(bc + 1) * 128, :], in_=res[:, :])

    nc.compile()
    return nc


def _get_program():
    global _PROGRAM
    if _PROGRAM is None:
        _PROGRAM = _build_program()
    return _PROGRAM


def _pad_edge(a):
    return np.concatenate([a[:, :1], a, a[:, -1:]], axis=1)


def _prep_core(sentences, pos1, pos2, masks, we8, pf18, pf28):
    """Per-core input prep. Returns xa, xb, dm (all E4 uint8-backed arrays)."""
    tok = _pad_edge(sentences)        # [NS, LP]
    p1 = _pad_edge(pos1)
    p2 = _pad_edge(pos2)
    # X: [NS, LP, IN_CH] fp8 (as uint8 for speed)
    X = np.zeros((NS, LP, IN_CH), np.uint8)
    X[:, :, :WD] = we8.view(np.uint8).reshape(VOCAB, WD)[tok.reshape(-1)] \
        .reshape(NS, LP, WD)
    X[:, :, WD:WD + PD] = pf18.view(np.uint8).reshape(2 * L, PD)[p1.reshape(-1)] \
        .reshape(NS, LP, PD)
    X[:, :, WD + PD:] = pf28.view(np.uint8).reshape(2 * L, PD)[p2.reshape(-1)] \
        .reshape(NS, LP, PD)
    # channel-major interleaved stream: col = sg*512 + 4*token + sent
    buf = np.zeros((NBLK, IN_CH, SG_PER_BLK, SG_COLS), np.uint8)
    Xb = X.reshape(NBLK, SG_PER_BLK, SGS, LP, IN_CH).transpose(0, 4, 1, 3, 2)
    buf[:, :, :, :SGS * LP] = Xb.reshape(NBLK, IN_CH, SG_PER_BLK, LP * SGS)
    buf = buf.reshape(NBLK, IN_CH, BLK_COLS)

    xa = buf[:, :256].reshape(NBLK, 2, 128, BLK_COLS).transpose(0, 2, 1, 3).copy()

    xb = np.zeros((NBLK, KB, 2, BLK_COLS), np.uint8)
    one8 = np.float32(1.0).astype(E4).view(np.uint8)
    # mask m0 row content: value 1.0 at stream col 4*(l+1) + s (center tap)
    m0 = masks[:, 0, :]  # [NS, L] (0/1 float)
    m0row = np.zeros((NBLK, SG_PER_BLK, SG_COLS), np.uint8)
    m0v = m0row[:, :, :SGS * LP].reshape(NBLK, SG_PER_BLK, LP, SGS)
    m0v[:, :, 1:L + 1, :] = \
        (m0.reshape(NBLK, SG_PER_BLK, SGS, L).transpose(0, 1, 3, 2) > 0.5) * one8
    m0row = m0row.reshape(NBLK, BLK_COLS)
    xb[:, :NPAIR_B, 0, :] = buf[:, 256:256 + NPAIR_B]
    xb[:, :NPAIR_B, 1, :] = buf[:, 256 + NPAIR_B:256 + 2 * NPAIR_B]
    xb[:, NPAIR_B, 0, :] = m0row

    # mask diff rows: 128*(m1-m0), 128*(m2-m1) at [blk, j, s_local*120 + l]
    # mask diff rows at interleaved cols: dm[blk, row, sg*480 + 4*l + s]
    d1 = (masks[:, 1, :] - masks[:, 0, :]) * MB
    d2 = (masks[:, 2, :] - masks[:, 1, :]) * MB
    dd = np.stack([d1, d2], axis=1).astype(np.float32) \
        .reshape(NBLK, SG_PER_BLK, SGS, 2, L).transpose(0, 3, 1, 4, 2) \
        .reshape(NBLK, 1, 2 * BLK * L)
    dm = dd.astype(E4)
    return xa.view(E4), xb.view(E4), dm


def _prep_shared(conv_w, conv_b, dense_w, dense_b, bag_ids, masks_unused=None):
    w8 = conv_w.astype(E4).astype(np.float32)  # quantize once
    wa = np.zeros((128, 2, 3, 2, 128), np.float32)
    wb = np.zeros((KB, 2, 3, 2, 128), np.float32)
    for fc, (f0, fw) in enumerate(FCH):
        for tap in range(3):
            for q in range(2):
                # wa[p, q, tap, fc, f] = w[f0+f, 128q+p, tap]
                wa[:, q, tap, fc, :fw] = w8[f0:f0 + fw, 128 * q:128 * (q + 1), tap].T
                wb[:NPAIR_B, q, tap, fc, :fw] = \
                    w8[f0:f0 + fw, 256 + NPAIR_B * q:256 + NPAIR_B * (q + 1), tap].T
        wb[NPAIR_B, 0, 1, fc, :fw] = MB  # mask rides center tap
    wa = wa.astype(E4)
    wb = wb.astype(E4)

    actb = np.zeros((128, 2), np.float32)
    for fc, (f0, fw) in enumerate(FCH):
        actb[:fw, fc] = conv_b[f0:f0 + fw] - MB

    dwt = np.zeros((128, 6 * NREL), np.float32)
    for j in range(3):
        for fc, (f0, fw) in enumerate(FCH):
            dwt[:fw, (j * 2 + fc) * NREL:(j * 2 + fc + 1) * NREL] = \
                dense_w[:, j * NF + f0:j * NF + f0 + fw].T
    dwt = dwt.astype(BF16)
    dbias = (dense_b / NCORES).reshape(1, NREL).astype(BF16)

    counts = np.bincount(bag_ids, minlength=NBAGS).astype(np.float32)
    counts = np.maximum(counts, 1.0)
    return wa, wb, actb, dwt, dbias, counts


def kernel(**inputs):
    sentences = np.asarray(inputs["sentences"]).astype(np.int32)
    pos1 = np.asarray(inputs["pos1"]).astype(np.int32)
    pos2 = np.asarray(inputs["pos2"]).astype(np.int32)
    masks = np.asarray(inputs["masks"]).astype(np.float32)
    bag_ids = np.asarray(inputs["bag_ids"]).astype(np.int64)
    word_emb = np.asarray(inputs["word_emb"]).astype(np.float32)
    pf1_emb = np.asarray(inputs["pf1_emb"]).astype(np.float32)
    pf2_emb = np.asarray(inputs["pf2_emb"]).astype(np.float32)
    conv_w = np.asarray(inputs["conv_w"]).astype(np.float32)
    conv_b = np.asarray(inputs["conv_b"]).astype(np.float32)
    dense_w = np.asarray(inputs["dense_w"]).astype(np.float32)
    dense_b = np.asarray(inputs["dense_b"]).astype(np.float32)

    we8 = word_emb.astype(E4)
    pf18 = pf1_emb.astype(E4)
    pf28 = pf2_emb.astype(E4)

    wa, wb, actb, dwt, dbias, counts = _prep_shared(
        conv_w, conv_b, dense_w, dense_b, bag_ids)

    in_maps = []
    for r in range(NCORES):
        sl = slice(r * NS, (r + 1) * NS)
        xa, xb, dm = _prep_core(sentences[sl], pos1[sl], pos2[sl], masks[sl],
                                we8, pf18, pf28)
        bags = bag_ids[sl]
        snorm = np.zeros((NS, NBAGS), np.float32)
        snorm[np.arange(NS), bags] = 1.0 / counts[bags]
        snorm = snorm.astype(BF16)
        in_maps.append({
            "xa": xa, "xb": xb, "dm": dm,
            "wa": wa, "wb": wb,
            "snorm": snorm, "dwt": dwt, "actb": actb, "dbias": dbias,
        })

    nc = _get_program()
    from concourse.bass_utils import run_bass_kernel_spmd

    trace = bool(int(os.environ.get("KERNEL_TRACE", "0")))
    res = run_bass_kernel_spmd(
        nc, in_maps, core_ids=list(range(NCORES)), trace=trace
    )
    global LAST_RESULT
    LAST_RESULT = res
    return res.results[0]["out"].astype(np.float32)


def _selftest():
    """Numpy-emulate the device program from the prepared arrays."""
    os.environ["JAX_PLATFORMS"] = "cpu"
    sys.path.insert(0, os.path.dirname(os.path.abspath(__file__)))
    import jax
    with jax.default_device(jax.devices("cpu")[0]):
        import reference
        inputs = reference.setup_inputs()
        expected = np.asarray(reference.reference(**inputs))
        inputs = {k: np.asarray(v) for k, v in inputs.items()}

    masks = inputs["masks"].astype(np.float32)
    bag_ids = inputs["bag_ids"].astype(np.int64)
    we8 = inputs["word_emb"].astype(np.float32).astype(E4)
    pf18 = inputs["pf1_emb"].astype(np.float32).astype(E4)
    pf28 = inputs["pf2_emb"].astype(np.float32).astype(E4)
    wa, wb, actb, dwt, dbias, counts = _prep_shared(
        inputs["conv_w"].astype(np.float32), inputs["conv_b"].astype(np.float32),
        inputs["dense_w"].astype(np.float32), inputs["dense_b"].astype(np.float32),
        bag_ids)
    waf = wa.astype(np.float32)
    wbf = wb.astype(np.float32)

    out_all = np.zeros((NBAGS, NREL), np.float32)
    for r in range(NCORES):
        sl = slice(r * NS, (r + 1) * NS)
        xa, xb, dm = _prep_core(
            inputs["sentences"].astype(np.int32)[sl],
            inputs["pos1"].astype(np.int32)[sl], inputs["pos2"].astype(np.int32)[sl],
            masks[sl], we8, pf18, pf28)
        xaf = xa.astype(np.float32)
        xbf = xb.astype(np.float32)
        dmf = dm.astype(np.float32)
        pooled = np.zeros((2, 128, 3, NS), np.float32)
        for blk in range(NBLK):
            for fc, (f0, fw) in enumerate(FCH):
                for grp in range(2):
                    ps = np.zeros((fw, 4, 4, L), np.float32)
                    for u in range(4):
                        sg = grp * 4 + u
                        for tap in range(3):
                            # rhs[p, q, s, l] = xa[blk, p, q, sg*512 + 4*(l+tap) + s]
                            cols = sg * SG_COLS + SGS * tap \
                                + np.arange(SGS)[:, None] \
                                + SGS * np.arange(L)[None, :]
                            rhs = xaf[blk][:, :, cols]          # [128, 2, 4, L]
                            lhs = waf[:, :, tap, fc, :fw]       # [128, 2, fw]
                            ps[:, u] += np.einsum('pqf,pqsl->fsl', lhs, rhs)
                            rhsb = xbf[blk][:, :, cols]
                            lhsb = wbf[:, :, tap, fc, :fw]
                            ps[:, u] += np.einsum('pqf,pqsl->fsl', lhsb, rhsb)
                    s0 = blk * BLK + grp * 16
                    for j in range(3):
                        pooled[fc, :fw, j, s0:s0 + 16] = \
                            ps.max(axis=3).transpose(0, 1, 2).reshape(fw, 16)
                        if j < 2:
                            for u in range(4):
                                sg = grp * 4 + u
                                add = dmf[blk, 0, j * BLK * L + sg * SGS * L:
                                          j * BLK * L + (sg + 1) * SGS * L] \
                                    .reshape(L, SGS).T
                                ps[:, u] += add[None, :, :]
        # tail
        pr = np.zeros((2, 128, 3, NS), np.float32)
        for fc in range(2):
            pr[fc] = np.maximum(pooled[fc] + actb[:, fc][:, None, None], 0)
        pr = pr.astype(BF16).astype(np.float32)
        dwtf = dwt.astype(np.float32)
        lg = np.zeros((NREL, NS), np.float32)
        for j in range(3):
            for fc, (f0, fw) in enumerate(FCH):
                lg += dwtf[:fw, (j * 2 + fc) * NREL:(j * 2 + fc + 1) * NREL].T @ \
                    pr[fc, :fw, j, :]
        lg = lg.astype(BF16).astype(np.float32)
        bags = bag_ids[sl]
        snorm = np.zeros((NS, NBAGS), np.float32)
        snorm[np.arange(NS), bags] = 1.0 / counts[bags]
        snorm = snorm.astype(BF16).astype(np.float32)
        out_all += snorm.T @ lg.T + dbias.astype(np.float32)

    e = np.exp(out_all - out_all.max(1, keepdims=True))
    sm = e / e.sum(1, keepdims=True)
    err = np.abs(sm - expected).max() / np.abs(expected).max()
    print("selftest rel err:", err)
    return err


if __name__ == "__main__":
    if "--selftest" in sys.argv:
        _selftest()


# revision 29
# speedup vs baseline: 1.1963x; 1.0446x over previous
"""Trainium2 Bass kernel for the PCNN (piecewise-CNN) bag-classification model.

V2 design (data-parallel over sentences, 256 sentences/core):
  Host: embedding gather + channel-major fp8(e4m3) layout upload (no on-device
        gather/transpose at all).
  Device per block of 32 sentences:
    conv1d(k=3, edge-pad) as fp8 DoubleRow matmuls: channels 0..255 ride the
    pair axis (q) of 3 full-K DR matmuls (one per tap); channels 256..309 + the
    piece-0 mask row ride a row-tiled triple (3 concurrent 28-pair DR matmuls,
    one per tap, at partition bases 0/32/64).
    PCNN piecewise max-pool: mask bias +128 (fp8-exact); j0 mask rides the conv
    contraction; j1/j2 are rank-1 fp8 matmul adds into PSUM; the three phase
    maxima come from 4-unit-batched DVE reduce_max over 4 PSUM banks.
  Tail: ReLU(+bias-128), dense to 53 logits, PE transpose, bag segment-mean as
        matmul with host-built normalized selection matrix, AllReduce, softmax.
"""

import os
import sys

for _p in ("/opt/trn_rl_repo",):
    if _p not in sys.path:
        sys.path.insert(0, _p)

import numpy as np
import ml_dtypes

# ---------------- problem constants (hardcoded per spec) ----------------
N = 2048          # total sentences
L = 120           # max sentence length
LP = 122          # edge-padded length
NCORES = 8
NS = N // NCORES  # 256 sentences per core
BLK = 32          # sentences per block
NBLK = NS // BLK  # 8 blocks
SGS = 4           # sentences per matmul unit
SG_PER_BLK = BLK // SGS          # 8
SG_COLS = 512                    # padded columns per unit (4*122=488 real)
BLK_COLS = SG_PER_BLK * SG_COLS  # 4096
NF = 230
NREL = 53
NBAGS = 256
VOCAB = 100000
WD = 300
PD = 5
IN_CH = WD + 2 * PD   # 310
FCH = [(0, 128), (128, 102)]   # filter chunks
MB = 128.0            # mask bias (fp8-exact)
NPAIR_B = 27          # channel pairs in chunk B (ch 256..309)
KB = NPAIR_B + 1      # + mask row

E4 = ml_dtypes.float8_e4m3fn
BF16 = ml_dtypes.bfloat16

_PROGRAM = None
LAST_RESULT = None


def _build_program():
    import concourse.bass as bass
    import concourse.mybir as mybir
    import concourse.tile as tile
    from concourse import bacc
    from concourse.masks import make_identity

    f32 = mybir.dt.float32
    bf16 = mybir.dt.bfloat16
    fp8 = mybir.dt.float8e4
    AF = mybir.ActivationFunctionType
    AX = mybir.AxisListType
    DR = mybir.MatmulPerfMode.DoubleRow

    nc = bacc.Bacc(
        "TRN2",
        target_bir_lowering=False,
        debug=False,
        num_devices=NCORES,
    )

    # ------------- external I/O -------------
    xa_d = nc.dram_tensor("xa", [NBLK, 128, 2, BLK_COLS], fp8, kind="ExternalInput").ap()
    xb_d = nc.dram_tensor("xb", [NBLK, KB, 2, BLK_COLS], fp8, kind="ExternalInput").ap()
    dm_d = nc.dram_tensor("dm", [NBLK, 1, 2 * BLK * L], fp8, kind="ExternalInput").ap()
    wa_d = nc.dram_tensor("wa", [128, 2, 3, 2, 128], fp8, kind="ExternalInput").ap()
    wb_d = nc.dram_tensor("wb", [KB, 2, 3, 2, 128], fp8, kind="ExternalInput").ap()
    snorm_d = nc.dram_tensor("snorm", [NS, NBAGS], bf16, kind="ExternalInput").ap()
    dwt_d = nc.dram_tensor("dwt", [128, 6 * NREL], bf16, kind="ExternalInput").ap()
    actb_d = nc.dram_tensor("actb", [128, 2], f32, kind="ExternalInput").ap()
    dbias_d = nc.dram_tensor("dbias", [1, NREL], bf16, kind="ExternalInput").ap()
    out_d = nc.dram_tensor("out", [NBAGS, NREL], f32, kind="ExternalOutput").ap()

    with tile.TileContext(nc) as tc:
        import contextlib

        ctx = contextlib.ExitStack()
        with ctx:
            singles = ctx.enter_context(tc.tile_pool(name="singles", bufs=1))

            # persistent tiles
            wa_sb = singles.tile([128, 2, 3, 2, 128], fp8, name="wa")
            wb_sb = singles.tile([KB, 2, 3, 2, 128], fp8, name="wb")
            snorm_sb = [singles.tile([128, NBAGS], bf16, name=f"sn{c}") for c in range(2)]
            dwt_sb = singles.tile([128, 6 * NREL], bf16)
            actb_sb = singles.tile([128, 2], f32)
            dbias_sb = singles.tile([1, NREL], bf16)
            ident = singles.tile([128, 128], bf16)
            ones_sb = singles.tile([1, 128], bf16)
            ones8 = singles.tile([1, 128], fp8)
            pooled = [singles.tile([128, 3, NS], f32, name=f"pool{c}") for c in range(2)]

            nc.sync.dma_start(out=wa_sb[:, :, :, :, :], in_=wa_d[:, :, :, :, :])
            nc.sync.dma_start(out=wb_sb[:, :, :, :, :], in_=wb_d[:, :, :, :, :])
            for c in range(2):
                nc.sync.dma_start(out=snorm_sb[c][:, :], in_=snorm_d[c * 128:(c + 1) * 128, :])
            nc.sync.dma_start(out=dwt_sb[:, :], in_=dwt_d[:, :])
            nc.sync.dma_start(out=actb_sb[:, :], in_=actb_d[:, :])
            nc.sync.dma_start(out=dbias_sb[:, :], in_=dbias_d[:, :])
            make_identity(nc, ident[:, :])
            nc.vector.memset(ones_sb[:, :], 1.0)
            nc.vector.memset(ones8[:, :], 1.0)
            nc.vector.memset(pooled[0][:, :, :], 0.0)
            nc.vector.memset(pooled[1][:, :, :], 0.0)

            xa_pool = ctx.enter_context(tc.tile_pool(name="xa", bufs=2))
            xb_pool = ctx.enter_context(tc.tile_pool(name="xb", bufs=2))
            dm_pool = ctx.enter_context(tc.tile_pool(name="dm", bufs=2))
            cv_psum = ctx.enter_context(tc.tile_pool(name="cv", bufs=2, space="PSUM"))

            UL = SGS * L  # 480 interleaved output columns per unit

            def emit_conv(xa, xb, fc, fw, grp):
                ps = cv_psum.tile([128, 4, 512], f32, tag="cv", name=f"cv{fc}_{grp}")
                # ---- conv: 3 full DR streams per unit ----
                for tap in range(3):
                    lhsA = wa_sb[:, :, tap, fc, 0:fw]
                    for u in range(4):
                        sg = grp * 4 + u
                        base = xa[0:128, 0:2, sg * SG_COLS + SGS * tap:
                                  sg * SG_COLS + SGS * tap + 1]
                        rhs = bass.AP(
                            tensor=base.tensor, offset=base.offset,
                            ap=[base.ap[0], [BLK_COLS, 2], [1, UL]],
                        )
                        nc.tensor.matmul(
                            out=ps[0:fw, u, 0:UL],
                            lhsT=lhsA,
                            rhs=rhs,
                            start=(tap == 0),
                            stop=False,
                            perf_mode=DR,
                            skip_group_check=True,
                        )
                # ---- chunk B: 3 tap streams per unit ----
                for t in range(3):
                    lhsB = wb_sb[0:KB, :, t, fc, 0:fw]
                    for u in range(4):
                        sg = grp * 4 + u
                        base = xb[0:KB, 0:2, sg * SG_COLS + SGS * t:
                                  sg * SG_COLS + SGS * t + 1]
                        rhs = bass.AP(
                            tensor=base.tensor, offset=base.offset,
                            ap=[base.ap[0], [BLK_COLS, 2], [1, UL]],
                        )
                        nc.tensor.matmul(
                            out=ps[0:fw, u, 0:UL],
                            lhsT=lhsB,
                            rhs=rhs,
                            start=False,
                            stop=False,
                            perf_mode=DR,
                            skip_group_check=True,
                        )
                return ps

            def emit_phases(st):
                ps, dm, blk, fc, fw, grp = st
                s0 = blk * BLK + grp * 16
                rbase = ps[0:fw, 0:4, 0:1]
                rin = bass.AP(
                    tensor=rbase.tensor, offset=rbase.offset,
                    ap=[rbase.ap[0], [512, 4], [1, SGS], [SGS, L]],
                )
                for j in range(3):
                    nc.vector.reduce_max(
                        out=pooled[fc][0:fw, j, s0:s0 + 16],
                        in_=rin,
                        axis=AX.X,
                    )
                    if j < 2:
                        for u in range(4):
                            sg = grp * 4 + u
                            nc.tensor.matmul(
                                out=ps[0:fw, u, 0:UL],
                                lhsT=ones8[0:1, 0:fw],
                                rhs=dm[0:1, j * BLK * L + sg * UL:
                                       j * BLK * L + (sg + 1) * UL],
                                start=False,
                                stop=(j == 1 and u == 3),
                                skip_group_check=True,
                            )

            pending = None
            for blk in range(NBLK):
                xa = xa_pool.tile([128, 2, BLK_COLS], fp8, tag="xa")
                xb = xb_pool.tile([KB, 2, BLK_COLS], fp8, tag="xb")
                dm = dm_pool.tile([1, 2 * BLK * L], fp8, tag="dm")
                nc.sync.dma_start(out=xa[:, :, :], in_=xa_d[blk, :, :, :])
                nc.sync.dma_start(out=xb[:, :, :], in_=xb_d[blk, :, :, :])
                nc.sync.dma_start(out=dm[:, :], in_=dm_d[blk, :, :])

                for fc, (f0, fw) in enumerate(FCH):
                    for grp in range(2):  # 4 units each
                        ps = emit_conv(xa, xb, fc, fw, grp)
                        if pending is not None:
                            emit_phases(pending)
                        pending = (ps, dm, blk, fc, fw, grp)
            emit_phases(pending)
            pending = None

            # ---------------- tail ----------------
            pr = [singles.tile([128, 3, NS], bf16, name=f"pr{c}") for c in range(2)]
            for fc in range(2):
                nc.scalar.activation(
                    out=pr[fc][:, :, :],
                    in_=pooled[fc][:, :, :],
                    func=AF.Relu,
                    bias=actb_sb[:, fc:fc + 1],
                    scale=1.0,
                )

            # dense: logitsT [53, 256] = sum_{j,fc} dwt[(j,fc)].T @ pr
            lg_ps = cv_psum.tile([NREL, NS], f32, tag="cv", name="lgps")
            nmm = 0
            for j in range(3):
                for fc, (f0, fw) in enumerate(FCH):
                    nc.tensor.matmul(
                        out=lg_ps[:, :],
                        lhsT=dwt_sb[0:fw, (j * 2 + fc) * NREL:(j * 2 + fc + 1) * NREL],
                        rhs=pr[fc][0:fw, j, :],
                        start=(nmm == 0),
                        stop=(nmm == 5),
                    )
                    nmm += 1
            lg_sb = singles.tile([NREL, NS], bf16)
            nc.vector.tensor_copy(out=lg_sb[:, :], in_=lg_ps[:, :])

            # transpose logits -> [256 sents, 53]
            ls = [singles.tile([128, NREL], bf16, name=f"ls{c}") for c in range(2)]
            for sc in range(2):
                ltp = cv_psum.tile([128, 128], bf16, tag="cv", name="ltp")
                nc.tensor.transpose(
                    out=ltp[0:128, 0:NREL],
                    in_=lg_sb[:, sc * 128:(sc + 1) * 128],
                    identity=ident[0:NREL, 0:NREL],
                )
                nc.vector.tensor_copy(out=ls[sc][:, :], in_=ltp[0:128, 0:NREL])

            # bag aggregation: bagT [128 bags, 53] per bag-chunk (+ dense bias/8)
            cc_dram = ctx.enter_context(tc.tile_pool(name="ccd", bufs=1, space="DRAM"))
            cc_in = cc_dram.tile([NBAGS, NREL], f32)
            cc_out = cc_dram.tile([NBAGS, NREL], f32)
            for bc in range(2):
                bg = cv_psum.tile([128, NREL], f32, tag="cv", name="bg")
                for sc in range(2):
                    nc.tensor.matmul(
                        out=bg[:, :],
                        lhsT=snorm_sb[sc][:, bc * 128:(bc + 1) * 128],
                        rhs=ls[sc][:, :],
                        start=(sc == 0),
                        stop=False,
                    )
                nc.tensor.matmul(
                    out=bg[:, :],
                    lhsT=ones_sb[0:1, 0:128],
                    rhs=dbias_sb[0:1, :],
                    start=False,
                    stop=True,
                )
                bg_sb = singles.tile([128, NREL], f32, name=f"bgs{bc}")
                nc.vector.tensor_copy(out=bg_sb[:, :], in_=bg[:, :])
                nc.sync.dma_start(out=cc_in[bc * 128:(bc + 1) * 128, :], in_=bg_sb[:, :])

            nc.gpsimd.collective_compute(
                "AllReduce",
                mybir.AluOpType.add,
                replica_groups=[list(range(NCORES))],
                ins=[cc_in.opt()],
                outs=[cc_out.opt()],
            )

            # softmax over the 53 relations
            for bc in range(2):
                t = singles.tile([128, NREL], f32, name=f"sm{bc}")
                nc.sync.dma_start(out=t[:, :], in_=cc_out[bc * 128:(bc + 1) * 128, :])
                nmax = singles.tile([128, 1], f32, name=f"nmax{bc}")
                nc.vector.reduce_max(out=nmax[:, :], in_=t[:, :], axis=AX.X, negate=True)
                ex = singles.tile([128, NREL], f32, name=f"ex{bc}")
                nc.scalar.activation(
                    out=ex[:, :], in_=t[:, :], func=AF.Exp, bias=nmax[:, :], scale=1.0
                )
                ssum = singles.tile([128, 1], f32, name=f"ssum{bc}")
                nc.vector.reduce_sum(out=ssum[:, :], in_=ex[:, :], axis=AX.X)
                rcp = singles.tile([128, 1], f32, name=f"rcp{bc}")
                nc.vector.reciprocal(out=rcp[:, :], in_=ssum[:, :])
                res = singles.tile([128, NREL], f32, name=f"res{bc}")
                nc.vector.tensor_scalar_mul(res[:, :], ex[:, :], rcp[:, :])
                nc.sync.dma_start(out=out_d[bc * 128:(bc + 1) * 128, :], in_=res[:, :])

    nc.compile()
    return nc


def _get_program():
    global _PROGRAM
    if _PROGRAM is None:
        _PROGRAM = _build_program()
    return _PROGRAM


def _pad_edge(a):
    return np.concatenate([a[:, :1], a, a[:, -1:]], axis=1)


def _prep_core(sentences, pos1, pos2, masks, we8, pf18, pf28):
    """Per-core input prep. Returns xa, xb, dm (all E4 uint8-backed arrays)."""
    tok = _pad_edge(sentences)        # [NS, LP]
    p1 = _pad_edge(pos1)
    p2 = _pad_edge(pos2)
    # X: [NS, LP, IN_CH] fp8 (as uint8 for speed)
    X = np.zeros((NS, LP, IN_CH), np.uint8)
    X[:, :, :WD] = we8.view(np.uint8).reshape(VOCAB, WD)[tok.reshape(-1)] \
        .reshape(NS, LP, WD)
    X[:, :, WD:WD + PD] = pf18.view(np.uint8).reshape(2 * L, PD)[p1.reshape(-1)] \
        .reshape(NS, LP, PD)
    X[:, :, WD + PD:] = pf28.view(np.uint8).reshape(2 * L, PD)[p2.reshape(-1)] \
        .reshape(NS, LP, PD)
    # channel-major interleaved stream: col = sg*512 + 4*token + sent
    buf = np.zeros((NBLK, IN_CH, SG_PER_BLK, SG_COLS), np.uint8)
    Xb = X.reshape(NBLK, SG_PER_BLK, SGS, LP, IN_CH).transpose(0, 4, 1, 3, 2)
    buf[:, :, :, :SGS * LP] = Xb.reshape(NBLK, IN_CH, SG_PER_BLK, LP * SGS)
    buf = buf.reshape(NBLK, IN_CH, BLK_COLS)

    xa = buf[:, :256].reshape(NBLK, 2, 128, BLK_COLS).transpose(0, 2, 1, 3).copy()

    xb = np.zeros((NBLK, KB, 2, BLK_COLS), np.uint8)
    one8 = np.float32(1.0).astype(E4).view(np.uint8)
    # mask m0 row content: value 1.0 at stream col 4*(l+1) + s (center tap)
    m0 = masks[:, 0, :]  # [NS, L] (0/1 float)
    m0row = np.zeros((NBLK, SG_PER_BLK, SG_COLS), np.uint8)
    m0v = m0row[:, :, :SGS * LP].reshape(NBLK, SG_PER_BLK, LP, SGS)
    m0v[:, :, 1:L + 1, :] = \
        (m0.reshape(NBLK, SG_PER_BLK, SGS, L).transpose(0, 1, 3, 2) > 0.5) * one8
    m0row = m0row.reshape(NBLK, BLK_COLS)
    xb[:, :NPAIR_B, 0, :] = buf[:, 256:256 + NPAIR_B]
    xb[:, :NPAIR_B, 1, :] = buf[:, 256 + NPAIR_B:256 + 2 * NPAIR_B]
    xb[:, NPAIR_B, 0, :] = m0row

    # mask diff rows: 128*(m1-m0), 128*(m2-m1) at [blk, j, s_local*120 + l]
    # mask diff rows at interleaved cols: dm[blk, row, sg*480 + 4*l + s]
    d1 = (masks[:, 1, :] - masks[:, 0, :]) * MB
    d2 = (masks[:, 2, :] - masks[:, 1, :]) * MB
    dd = np.stack([d1, d2], axis=1).astype(np.float32) \
        .reshape(NBLK, SG_PER_BLK, SGS, 2, L).transpose(0, 3, 1, 4, 2) \
        .reshape(NBLK, 1, 2 * BLK * L)
    dm = dd.astype(E4)
    return xa.view(E4), xb.view(E4), dm


def _prep_shared(conv_w, conv_b, dense_w, dense_b, bag_ids, masks_unused=None):
    w8 = conv_w.astype(E4).astype(np.float32)  # quantize once
    wa = np.zeros((128, 2, 3, 2, 128), np.float32)
    wb = np.zeros((KB, 2, 3, 2, 128), np.float32)
    for fc, (f0, fw) in enumerate(FCH):
        for tap in range(3):
            for q in range(2):
                # wa[p, q, tap, fc, f] = w[f0+f, 128q+p, tap]
                wa[:, q, tap, fc, :fw] = w8[f0:f0 + fw, 128 * q:128 * (q + 1), tap].T
                wb[:NPAIR_B, q, tap, fc, :fw] = \
                    w8[f0:f0 + fw, 256 + NPAIR_B * q:256 + NPAIR_B * (q + 1), tap].T
        wb[NPAIR_B, 0, 1, fc, :fw] = MB  # mask rides center tap
    wa = wa.astype(E4)
    wb = wb.astype(E4)

    actb = np.zeros((128, 2), np.float32)
    for fc, (f0, fw) in enumerate(FCH):
        actb[:fw, fc] = conv_b[f0:f0 + fw] - MB

    dwt = np.zeros((128, 6 * NREL), np.float32)
    for j in range(3):
        for fc, (f0, fw) in enumerate(FCH):
            dwt[:fw, (j * 2 + fc) * NREL:(j * 2 + fc + 1) * NREL] = \
                dense_w[:, j * NF + f0:j * NF + f0 + fw].T
    dwt = dwt.astype(BF16)
    dbias = (dense_b / NCORES).reshape(1, NREL).astype(BF16)

    counts = np.bincount(bag_ids, minlength=NBAGS).astype(np.float32)
    counts = np.maximum(counts, 1.0)
    return wa, wb, actb, dwt, dbias, counts


def kernel(**inputs):
    sentences = np.asarray(inputs["sentences"]).astype(np.int32)
    pos1 = np.asarray(inputs["pos1"]).astype(np.int32)
    pos2 = np.asarray(inputs["pos2"]).astype(np.int32)
    masks = np.asarray(inputs["masks"]).astype(np.float32)
    bag_ids = np.asarray(inputs["bag_ids"]).astype(np.int64)
    word_emb = np.asarray(inputs["word_emb"]).astype(np.float32)
    pf1_emb = np.asarray(inputs["pf1_emb"]).astype(np.float32)
    pf2_emb = np.asarray(inputs["pf2_emb"]).astype(np.float32)
    conv_w = np.asarray(inputs["conv_w"]).astype(np.float32)
    conv_b = np.asarray(inputs["conv_b"]).astype(np.float32)
    dense_w = np.asarray(inputs["dense_w"]).astype(np.float32)
    dense_b = np.asarray(inputs["dense_b"]).astype(np.float32)

    we8 = word_emb.astype(E4)
    pf18 = pf1_emb.astype(E4)
    pf28 = pf2_emb.astype(E4)

    wa, wb, actb, dwt, dbias, counts = _prep_shared(
        conv_w, conv_b, dense_w, dense_b, bag_ids)

    in_maps = []
    for r in range(NCORES):
        sl = slice(r * NS, (r + 1) * NS)
        xa, xb, dm = _prep_core(sentences[sl], pos1[sl], pos2[sl], masks[sl],
                                we8, pf18, pf28)
        bags = bag_ids[sl]
        snorm = np.zeros((NS, NBAGS), np.float32)
        snorm[np.arange(NS), bags] = 1.0 / counts[bags]
        snorm = snorm.astype(BF16)
        in_maps.append({
            "xa": xa, "xb": xb, "dm": dm,
            "wa": wa, "wb": wb,
            "snorm": snorm, "dwt": dwt, "actb": actb, "dbias": dbias,
        })

    nc = _get_program()
    from concourse.bass_utils import run_bass_kernel_spmd

    trace = bool(int(os.environ.get("KERNEL_TRACE", "0")))
    res = run_bass_kernel_spmd(
        nc, in_maps, core_ids=list(range(NCORES)), trace=trace
    )
    global LAST_RESULT
    LAST_RESULT = res
    return res.results[0]["out"].astype(np.float32)


def _selftest():
    """Numpy-emulate the device program from the prepared arrays."""
    os.environ["JAX_PLATFORMS"] = "cpu"
    sys.path.insert(0, os.path.dirname(os.path.abspath(__file__)))
    import jax
    with jax.default_device(jax.devices("cpu")[0]):
        import reference
        inputs = reference.setup_inputs()
        expected = np.asarray(reference.reference(**inputs))
        inputs = {k: np.asarray(v) for k, v in inputs.items()}

    masks = inputs["masks"].astype(np.float32)
    bag_ids = inputs["bag_ids"].astype(np.int64)
    we8 = inputs["word_emb"].astype(np.float32).astype(E4)
    pf18 = inputs["pf1_emb"].astype(np.float32).astype(E4)
    pf28 = inputs["pf2_emb"].astype(np.float32).astype(E4)
    wa, wb, actb, dwt, dbias, counts = _prep_shared(
        inputs["conv_w"].astype(np.float32), inputs["conv_b"].astype(np.float32),
        inputs["dense_w"].astype(np.float32), inputs["dense_b"].astype(np.float32),
        bag_ids)
    waf = wa.astype(np.float32)
    wbf = wb.astype(np.float32)

    out_all = np.zeros((NBAGS, NREL), np.float32)
    for r in range(NCORES):
        sl = slice(r * NS, (r + 1) * NS)
        xa, xb, dm = _prep_core(
            inputs["sentences"].astype(np.int32)[sl],
            inputs["pos1"].astype(np.int32)[sl], inputs["pos2"].astype(np.int32)[sl],
            masks[sl], we8, pf18, pf28)
        xaf = xa.astype(np.float32)
        xbf = xb.astype(np.float32)
        dmf = dm.astype(np.float32)
        pooled = np.zeros((2, 128, 3, NS), np.float32)
        for blk in range(NBLK):
            for fc, (f0, fw) in enumerate(FCH):
                for grp in range(2):
                    ps = np.zeros((fw, 4, 4, L), np.float32)
                    for u in range(4):
                        sg = grp * 4 + u
                        for tap in range(3):
                            # rhs[p, q, s, l] = xa[blk, p, q, sg*512 + 4*(l+tap) + s]
                            cols = sg * SG_COLS + SGS * tap \
                                + np.arange(SGS)[:, None] \
                                + SGS * np.arange(L)[None, :]
                            rhs = xaf[blk][:, :, cols]          # [128, 2, 4, L]
                            lhs = waf[:, :, tap, fc, :fw]       # [128, 2, fw]
                            ps[:, u] += np.einsum('pqf,pqsl->fsl', lhs, rhs)
                            rhsb = xbf[blk][:, :, cols]
                            lhsb = wbf[:, :, tap, fc, :fw]
                            ps[:, u] += np.einsum('pqf,pqsl->fsl', lhsb, rhsb)
                    s0 = blk * BLK + grp * 16
                    for j in range(3):
                        pooled[fc, :fw, j, s0:s0 + 16] = \
                            ps.max(axis=3).transpose(0, 1, 2).reshape(fw, 16)
                        if j < 2:
                            for u in range(4):
                                sg = grp * 4 + u
                                add = dmf[blk, 0, j * BLK * L + sg * SGS * L:
                                          j * BLK * L + (sg + 1) * SGS * L] \
                                    .reshape(L, SGS).T
                                ps[:, u] += add[None, :, :]
        # tail
        pr = np.zeros((2, 128, 3, NS), np.float32)
        for fc in range(2):
            pr[fc] = np.maximum(pooled[fc] + actb[:, fc][:, None, None], 0)
        pr = pr.astype(BF16).astype(np.float32)
        dwtf = dwt.astype(np.float32)
        lg = np.zeros((NREL, NS), np.float32)
        for j in range(3):
            for fc, (f0, fw) in enumerate(FCH):
                lg += dwtf[:fw, (j * 2 + fc) * NREL:(j * 2 + fc + 1) * NREL].T @ \
                    pr[fc, :fw, j, :]
        lg = lg.astype(BF16).astype(np.float32)
        bags = bag_ids[sl]
        snorm = np.zeros((NS, NBAGS), np.float32)
        snorm[np.arange(NS), bags] = 1.0 / counts[bags]
        snorm = snorm.astype(BF16).astype(np.float32)
        out_all += snorm.T @ lg.T + dbias.astype(np.float32)

    e = np.exp(out_all - out_all.max(1, keepdims=True))
    sm = e / e.sum(1, keepdims=True)
    err = np.abs(sm - expected).max() / np.abs(expected).max()
    print("selftest rel err:", err)
    return err


if __name__ == "__main__":
    if "--selftest" in sys.argv:
        _selftest()
